# revision 25
# baseline (speedup 1.0000x reference)
"""Trainium2 Bass kernel for nn_AttentionModule (batch-softmax attention + MLP).

Sharding: batch-parallel over the 8 NeuronCores (8 of 64 batches per core).
The softmax over dim=0 (batch) needs the cross-core sum of exp(scores): a
bf16 AllReduce of Z[h, m, n] per half-head (0.25 MB), pipelined against the
other half's score/exp compute.  Everything else is local.

v2 structure (per core, b = local batch 0..7, h = head 0..3):
  proj (token-major, all heads grouped per 128-token chunk):
    PQ|PV     = x_c @ [Wq|Wv] + bias      (512-free matmuls + rank-1 bias)
    PK        = x_c @ Wk + bias           (stats only -- never applied)
    stats     = bn_stats on PSUM (DVE), combined with elementwise ops
    rstd      = exp(-0.5*ln(var+eps))     (ACT, same table set as the
                softmax Exp; K's rstd also folds in the 1/11 score scale)
    Q,V       = (raw - mu) * rstd         (gpsimd, from bf16 SBUF copies)
    qt        = Q^T per head              (PE transpose, deferred one batch)
  attention (head-major, AllReduce per half-head):
    KT_raw    = Wk_h^T @ x_b^T + bk       (feature-major K: NO LayerNorm
                apply and NO transpose -- the LN'd Q is zero-mean along d,
                so K's mean subtraction cancels in the scores, and K's rstd
                is applied as the per-partition Exp scale)
    E^T[m,n]  = exp(rk[m]/11 * (KT_raw chunk)^T @ qt)
    Z-half    = sum_b E (DVE bf16 tree) -> bf16 AllReduce -> R = exp(-ln(Z))
    A^T       += V_c^T @ (E^T * R)        (PSUM accumulation, at += per half)
  mlp (per b): h1^T = relu(W1^T A_cat^T + b1); mlp = relu(h1 W2 + b2);
    out = LN(x + mlp)  (gamma/beta applied host-side; they are affine
    constants of the final LayerNorm)
"""

import sys

for _p in ("/opt/trn_rl_repo", "/opt/pypackages"):
    if _p not in sys.path:
        sys.path.append(_p)

from contextlib import ExitStack

import numpy as np

import concourse.bass as bass
import concourse.tile as tile
from concourse import bacc, masks, mybir
from concourse.bass_utils import run_bass_kernel_spmd

B, N, E, H = 64, 512, 128, 4
NCORES = 8
BL = B // NCORES          # local batches per core
NCH = N // 128            # 128-row chunks per sequence
EPS = 1e-5
NLN11 = -float(np.log(11.0))  # fold the 1/11 score scale into K's rstd

F32 = mybir.dt.float32
BF16 = mybir.dt.bfloat16
AF = mybir.ActivationFunctionType
ALU = mybir.AluOpType


def _reordered_act_tables(arch):
    """Activation tables filtered so one set serves every function we use.

    The table-load placement pass assigns each activation function to the
    first set that contains it, which splits Exp (exp_and_others) and Ln
    (natural_log) into different sets and makes the ACT engine ping-pong
    its spline tables (~1.3us per switch).  This kernel only uses
    {Identity, Copy, Exp, Ln, Relu}, all present in
    natural_log_exp_and_others; removing them from every other set forces
    the pass to pick that one set for all activations.  Set IDs are
    positional (index into act_info.json), so the original dict order is
    preserved.
    """
    from concourse.hw_specs import get_activation_tables

    tabs = get_activation_tables(arch)
    pref = "natural_log_exp_and_others"
    keep = tabs[pref]
    return {name: (fns if name == pref else (fns - keep))
            for name, fns in tabs.items()}


def _build():
    nc = bacc.Bacc(None, target_bir_lowering=False, debug=False)
    _orig_tables = bacc.get_activation_tables
    bacc.get_activation_tables = _reordered_act_tables

    x_ext = nc.declare_dram_parameter("x", [BL, N, E], F32, isOutput=False)
    xt_ext = nc.declare_dram_parameter("xt", [E, BL, N], BF16, isOutput=False)
    wq_ext = nc.declare_dram_parameter("wqt", [E, H, 128], BF16, isOutput=False)
    wk_ext = nc.declare_dram_parameter("wkt", [E, H, 128], BF16, isOutput=False)
    wv_ext = nc.declare_dram_parameter("wvt", [E, H, 128], BF16, isOutput=False)
    bq_ext = nc.declare_dram_parameter("bqr", [1, H, 128], BF16, isOutput=False)
    bk_ext = nc.declare_dram_parameter("bkr", [1, H, 128], BF16, isOutput=False)
    bv_ext = nc.declare_dram_parameter("bvr", [1, H, 128], BF16, isOutput=False)
    bkc_ext = nc.declare_dram_parameter("bkc", [128, H], F32, isOutput=False)
    w1_ext = nc.declare_dram_parameter("w1t", [E, H, 128], BF16, isOutput=False)
    w2_ext = nc.declare_dram_parameter("w2c", [128, 128], BF16, isOutput=False)
    b1_ext = nc.declare_dram_parameter("b1c", [128, 1], F32, isOutput=False)
    b2_ext = nc.declare_dram_parameter("b2r", [1, 128], BF16, isOutput=False)
    out_ext = nc.declare_dram_parameter("out", [BL, N, E], F32, isOutput=True)

    with tile.TileContext(nc) as tc:
        with ExitStack() as ctx:
            _emit(ctx, tc, x_ext, xt_ext, wq_ext, wk_ext, wv_ext, bq_ext,
                  bk_ext, bv_ext, bkc_ext, w1_ext, w2_ext, b1_ext, b2_ext,
                  out_ext)
    try:
        nc.compile()
    finally:
        bacc.get_activation_tables = _orig_tables
    return nc


def _emit(ctx, tc, x_ext, xt_ext, wq_ext, wk_ext, wv_ext, bq_ext, bk_ext,
          bv_ext, bkc_ext, w1_ext, w2_ext, b1_ext, b2_ext, out_ext):
    nc = tc.nc

    persist = ctx.enter_context(tc.tile_pool(name="persist", bufs=1))
    qn_pool = ctx.enter_context(tc.tile_pool(name="qn", bufs=2))
    qvr_pool = ctx.enter_context(tc.tile_pool(name="qvr", bufs=4))
    st_pool = ctx.enter_context(tc.tile_pool(name="st", bufs=2))
    stat = ctx.enter_context(tc.tile_pool(name="stat", bufs=10))
    expd_pool = ctx.enter_context(tc.tile_pool(name="expd", bufs=16))
    ztmp_pool = ctx.enter_context(tc.tile_pool(name="ztmp", bufs=2))
    lnz_pool = ctx.enter_context(tc.tile_pool(name="lnz", bufs=1))
    rr_pool = ctx.enter_context(tc.tile_pool(name="rr", bufs=3))
    ktb_pool = ctx.enter_context(tc.tile_pool(name="ktb", bufs=8))
    mlpp = ctx.enter_context(tc.tile_pool(name="mlpp", bufs=1))
    dram = ctx.enter_context(tc.tile_pool(name="dram", bufs=4, space="DRAM"))

    ps_big = ctx.enter_context(tc.tile_pool(name="ps_big", bufs=2,
                                            space="PSUM"))
    ps_small = ctx.enter_context(tc.tile_pool(name="ps_small", bufs=4,
                                              space="PSUM"))

    # ---- tiny dummy collective: pulls the one-time collective-init
    # barrier into the load phase ----
    dmy_in = dram.tile([128, 4], F32, tag="dmy_in")
    dmy_out = dram.tile([128, 4], F32, tag="dmy_out")
    zt = stat.tile([128, 4], F32, tag="zt")
    nc.vector.memset(zt[:], 0.0)
    nc.sync.dma_start(dmy_in[:], zt[:])
    replica = [list(range(NCORES))]
    nc.gpsimd.collective_compute("AllReduce", ALU.add,
                                 replica_groups=replica,
                                 ins=[dmy_in.opt()], outs=[dmy_out.opt()])

    # ---- constants & weights (all pre-laid-out host-side, direct DMA) ----
    ident = persist.tile([128, 128], BF16)
    masks.make_identity(nc, ident[:])
    ones1 = persist.tile([1, 128], BF16)
    nc.vector.memset(ones1[:], 1.0)
    epst = persist.tile([128, 1], F32)
    nc.vector.memset(epst[:], EPS)
    nln11t = persist.tile([128, 1], F32)
    nc.vector.memset(nln11t[:], NLN11)

    xt = persist.tile([128, BL, N], BF16)
    nc.sync.dma_start(xt[:], xt_ext[:])
    wq_sb = persist.tile([128, H, 128], BF16)
    nc.sync.dma_start(wq_sb[:], wq_ext[:])
    wk_sb = persist.tile([128, H, 128], BF16)
    nc.sync.dma_start(wk_sb[:], wk_ext[:])
    wv_sb = persist.tile([128, H, 128], BF16)
    nc.sync.dma_start(wv_sb[:], wv_ext[:])
    bq_sb = persist.tile([1, H, 128], BF16)
    nc.sync.dma_start(bq_sb[:], bq_ext[:])
    bk_sb = persist.tile([1, H, 128], BF16)
    nc.sync.dma_start(bk_sb[:], bk_ext[:])
    bv_sb = persist.tile([1, H, 128], BF16)
    nc.sync.dma_start(bv_sb[:], bv_ext[:])
    bkc_sb = persist.tile([128, H], F32)
    nc.sync.dma_start(bkc_sb[:], bkc_ext[:])
    w1_sb = persist.tile([128, H, 128], BF16)
    nc.sync.dma_start(w1_sb[:], w1_ext[:])
    w2_sb = persist.tile([128, 128], BF16)
    nc.sync.dma_start(w2_sb[:], w2_ext[:])
    b1col = persist.tile([128, 1], F32)
    nc.sync.dma_start(b1col[:], b1_ext[:])
    b2row = persist.tile([1, 128], BF16)
    nc.sync.dma_start(b2row[:], b2_ext[:])

    # ---- persistent per-core state ----
    qt_sb = persist.tile([128, BL, H, N], BF16)     # Q^T per (b, h)
    vh_sb = persist.tile([128, BL, NCH, H, 128], BF16)  # normalized V
    at_sb = persist.tile([128, BL, H, N], BF16)     # A^T per (b, h)
    rk_sb = persist.tile([128, BL, NCH, H], F32)    # K rstd / 11 columns

    # ================= projection phase =================
    pending_tp = None

    def _transposes(b, qn):
        for h in range(H):
            qtp = ps_small.tile([128, N], BF16, tag="small")
            for c in range(NCH):
                nc.tensor.transpose(qtp[:, c * 128:(c + 1) * 128],
                                    qn[:, h, c, :], ident[:])
            nc.vector.tensor_copy(qt_sb[:, b, h, :], qtp[:])

    for b in range(BL):
        qn = qn_pool.tile([128, H, NCH, 128], BF16, tag="qn", name=f"qn_{b}")
        st = st_pool.tile([128, NCH, 3, H, 6], F32, tag="st", name=f"st_{b}")
        raws = []
        for c in range(NCH):
            xt_c = xt[:, b, c * 128:(c + 1) * 128]
            ppqv = ps_big.tile([128, 2, H, 128], F32, tag="big")
            nc.tensor.matmul(ppqv[:, 0], xt_c, wq_sb[:], start=True,
                             stop=False)
            nc.tensor.matmul(ppqv[:, 0], ones1[:], bq_sb[:], start=False,
                             stop=True)
            nc.tensor.matmul(ppqv[:, 1], xt_c, wv_sb[:], start=True,
                             stop=False)
            nc.tensor.matmul(ppqv[:, 1], ones1[:], bv_sb[:], start=False,
                             stop=True)
            ppk = ps_small.tile([128, H, 128], F32, tag="small")
            nc.tensor.matmul(ppk[:], xt_c, wk_sb[:], start=True, stop=False)
            nc.tensor.matmul(ppk[:], ones1[:], bk_sb[:], start=False,
                             stop=True)
            for hh in range(H):
                nc.vector.bn_stats(st[:, c, 0, hh, :], ppqv[:, 0, hh, :])
                nc.vector.bn_stats(st[:, c, 2, hh, :], ppqv[:, 1, hh, :])
                nc.vector.bn_stats(st[:, c, 1, hh, :], ppk[:, hh, :])
            qvr = qvr_pool.tile([128, 2, H, 128], BF16, tag="qvr",
                                name=f"qvr_{b}_{c}")
            nc.scalar.activation(qvr[:], ppqv[:], AF.Copy)
            raws.append(qvr)

        if pending_tp is not None:
            pending_tp()
            pending_tp = None

        # combine bn_stats halves into mean / var (+eps via the Ln bias)
        mu = stat.tile([128, NCH, 3, H], F32, tag="mu", name=f"mu_{b}")
        nc.vector.tensor_add(mu[:], st[:, :, :, :, 1], st[:, :, :, :, 4])
        nc.vector.tensor_scalar(mu[:], mu[:], 0.5, None, op0=ALU.mult)
        dh = stat.tile([128, NCH, 3, H], F32, tag="dh", name=f"dh_{b}")
        nc.vector.tensor_sub(dh[:], st[:, :, :, :, 1], st[:, :, :, :, 4])
        nc.vector.tensor_scalar(dh[:], dh[:], 0.5, None, op0=ALU.mult)
        nc.vector.tensor_mul(dh[:], dh[:], dh[:])
        var = stat.tile([128, NCH, 3, H], F32, tag="var", name=f"var_{b}")
        nc.vector.tensor_add(var[:], st[:, :, :, :, 2], st[:, :, :, :, 5])
        nc.vector.scalar_tensor_tensor(var[:], var[:], 1.0 / 128.0, dh[:],
                                       op0=ALU.mult, op1=ALU.add)
        lnv = stat.tile([128, NCH, 3, H], F32, tag="lnv", name=f"lnv_{b}")
        nc.scalar.activation(lnv[:], var[:], AF.Ln, bias=epst[:])
        rstd = stat.tile([128, NCH, 3, H], F32, tag="rstd", name=f"rstd_{b}")
        nc.scalar.activation(rstd[:, :, 0, :], lnv[:, :, 0, :], AF.Exp,
                             scale=-0.5)
        nc.scalar.activation(rstd[:, :, 2, :], lnv[:, :, 2, :], AF.Exp,
                             scale=-0.5)
        # K's rstd lands directly in rk_sb with the 1/11 fold
        nc.scalar.activation(rk_sb[:, b, :, :], lnv[:, :, 1, :], AF.Exp,
                             scale=-0.5, bias=nln11t[:])

        # LayerNorm applies on gpsimd (SBUF->SBUF; frees ACT/DVE)
        for c in range(NCH):
            for h in range(H):
                nc.gpsimd.tensor_scalar(qn[:, h, c, :], raws[c][:, 0, h, :],
                                        mu[:, c, 0, h:h + 1],
                                        rstd[:, c, 0, h:h + 1],
                                        op0=ALU.subtract, op1=ALU.mult)
                nc.gpsimd.tensor_scalar(vh_sb[:, b, c, h, :],
                                        raws[c][:, 1, h, :],
                                        mu[:, c, 2, h:h + 1],
                                        rstd[:, c, 2, h:h + 1],
                                        op0=ALU.subtract, op1=ALU.mult)

        pending_tp = (lambda b=b, qn=qn: _transposes(b, qn))

    pending_tp()

    # ================= attention phase =================
    def _ktb(h):
        tiles = []
        for b in range(BL):
            ktp = ps_small.tile([128, N], F32, tag="small")
            nc.tensor.matmul(ktp[:], wk_sb[:, h, :], xt[:, b, :],
                             start=True, stop=True)
            kt = ktb_pool.tile([128, N], BF16, tag="ktb",
                               name=f"ktb_{h}_{b}")
            nc.scalar.activation(kt[:], ktp[:], AF.Identity,
                                 bias=bkc_sb[:, h:h + 1])
            tiles.append(kt)
        return tiles

    def _half(h, half, ktb):
        """scores + exp for one (head, m-half); returns exp tiles + launches AR."""
        eds = []
        for b in range(BL):
            ed = expd_pool.tile([128, 2, N], BF16, tag="expd",
                                name=f"ed_{h}_{half}_{b}")
            for ci in range(2):
                c = half * 2 + ci
                ss = ps_big.tile([128, N], F32, tag="big")
                nc.tensor.matmul(ss[:], ktb[b][:, c * 128:(c + 1) * 128],
                                 qt_sb[:, b, h, :], start=True, stop=True)
                nc.scalar.activation(ed[:, ci, :], ss[:], AF.Exp,
                                     scale=rk_sb[:, b, c, h:h + 1])
            eds.append(ed)
        # Z = sum_b exp : accumulated by the DMA engines (CCE add) on the
        # way into the AllReduce input buffer; costs no ACT/DVE time
        zin = dram.tile([128, 2, N], BF16, tag="zin", name=f"zin_{h}_{half}")
        zout = dram.tile([128, 2, N], BF16, tag="zout",
                         name=f"zout_{h}_{half}")
        for i in range(BL):
            nc.gpsimd.dma_start(zin[:], eds[i][:],
                                accum_op=(ALU.bypass if i == 0 else ALU.add))
        nc.gpsimd.collective_compute("AllReduce", ALU.add,
                                     replica_groups=replica,
                                     ins=[zin.opt()], outs=[zout.opt()])
        return eds, zout

    def _rr(h, half, zout):
        zg = ztmp_pool.tile([128, 2, N], BF16, tag="zg",
                            name=f"zg_{h}_{half}")
        nc.sync.dma_start(zg[:], zout[:])
        rr = rr_pool.tile([128, 2, N], BF16, tag="rr", name=f"rr_{h}_{half}")
        for ci in range(2):
            lnz = lnz_pool.tile([128, N], F32, tag="lnz",
                                name=f"lnz_{h}_{half}_{ci}")
            nc.scalar.activation(lnz[:], zg[:, ci, :], AF.Ln)
            nc.scalar.activation(rr[:, ci, :], lnz[:], AF.Exp, scale=-1.0)
        return rr

    def _pv(h, half, eds, rr, and_mlp=False):
        for b in range(BL):
            ed = eds[b]
            pa = ps_small.tile([128, N], F32, tag="small")
            for ci in range(2):
                c = half * 2 + ci
                nc.vector.tensor_mul(ed[:, ci, :], ed[:, ci, :],
                                     rr[:, ci, :])
                nc.tensor.matmul(pa[:], vh_sb[:, b, c, h, :], ed[:, ci, :],
                                 start=(ci == 0), stop=(ci == 1))
            if half == 0:
                nc.vector.tensor_copy(at_sb[:, b, h, :], pa[:])
            else:
                nc.vector.tensor_add(at_sb[:, b, h, :], at_sb[:, b, h, :],
                                     pa[:])
            if and_mlp:
                _mlp_b(b)

    def _mlp_b(b):
        p1 = ps_big.tile([128, N], F32, tag="big", name=f"p1_{b}")
        for hh in range(H):
            nc.tensor.matmul(p1[:], w1_sb[:, hh, :], at_sb[:, b, hh, :],
                             start=(hh == 0), stop=(hh == H - 1))
        h1t = mlpp.tile([128, N], BF16, tag="h1t", name=f"h1t_{b}")
        nc.scalar.activation(h1t[:], p1[:], AF.Relu, bias=b1col[:])
        xres = mlpp.tile([128, NCH, 128], F32, tag="xres", name=f"xres_{b}")
        nc.sync.dma_start(xres[:],
                          x_ext[b].rearrange("(c p) e -> p c e", p=128))
        p2 = ps_small.tile([128, NCH, 128], F32, tag="small", name=f"p2_{b}")
        for c in range(NCH):
            nc.tensor.matmul(p2[:, c, :], h1t[:, c * 128:(c + 1) * 128],
                             w2_sb[:], start=True, stop=False)
            nc.tensor.matmul(p2[:, c, :], ones1[:], b2row[:], start=False,
                             stop=True)
        ys = mlpp.tile([128, NCH, 128], BF16, tag="ys", name=f"ys_{b}")
        nc.scalar.activation(ys[:], p2[:], AF.Relu)
        ysum = mlpp.tile([128, NCH, 128], BF16, tag="ysum", name=f"ysum_{b}")
        nc.gpsimd.tensor_add(ysum[:], ys[:], xres[:])
        st8 = stat.tile([128, NCH, 6], F32, tag="st8", name=f"st8_{b}")
        for c in range(NCH):
            nc.vector.bn_stats(st8[:, c, :], ysum[:, c, :])
        mu8 = stat.tile([128, NCH], F32, tag="mu8", name=f"mu8_{b}")
        nc.vector.tensor_add(mu8[:], st8[:, :, 1], st8[:, :, 4])
        nc.vector.tensor_scalar(mu8[:], mu8[:], 0.5, None, op0=ALU.mult)
        dh8 = stat.tile([128, NCH], F32, tag="dh8", name=f"dh8_{b}")
        nc.vector.tensor_sub(dh8[:], st8[:, :, 1], st8[:, :, 4])
        nc.vector.tensor_scalar(dh8[:], dh8[:], 0.5, None, op0=ALU.mult)
        nc.vector.tensor_mul(dh8[:], dh8[:], dh8[:])
        var8 = stat.tile([128, NCH], F32, tag="var8", name=f"var8_{b}")
        nc.vector.tensor_add(var8[:], st8[:, :, 2], st8[:, :, 5])
        nc.vector.scalar_tensor_tensor(var8[:], var8[:], 1.0 / 128.0, dh8[:],
                                       op0=ALU.mult, op1=ALU.add)
        ln8 = stat.tile([128, NCH], F32, tag="ln8", name=f"ln8_{b}")
        nc.scalar.activation(ln8[:], var8[:], AF.Ln, bias=epst[:])
        rstd8 = stat.tile([128, NCH], F32, tag="rstd8", name=f"rstd8_{b}")
        nc.scalar.activation(rstd8[:], ln8[:], AF.Exp, scale=-0.5)
        yo = mlpp.tile([128, NCH, 128], F32, tag="yo", name=f"yo_{b}")
        for c in range(NCH):
            nc.gpsimd.tensor_scalar(yo[:, c, :], ysum[:, c, :],
                                    mu8[:, c:c + 1], rstd8[:, c:c + 1],
                                    op0=ALU.subtract, op1=ALU.mult)
        nc.sync.dma_start(out_ext[b].rearrange("(c p) e -> p c e", p=128),
                          yo[:])

    ktb = _ktb(0)
    for h in range(H):
        eds0, zout0 = _half(h, 0, ktb)
        eds1, zout1 = _half(h, 1, ktb)
        rr0 = _rr(h, 0, zout0)
        _pv(h, 0, eds0, rr0)
        if h < H - 1:
            ktb = _ktb(h + 1)
        rr1 = _rr(h, 1, zout1)
        _pv(h, 1, eds1, rr1, and_mlp=(h == H - 1))


_NC_CACHE = None


def make_in_maps(inputs):
    import ml_dtypes

    bf = ml_dtypes.bfloat16
    f = {k: np.asarray(v, dtype=np.float32) for k, v in inputs.items()}
    Wq, Wk, Wv, W1 = f["Wq"], f["Wk"], f["Wv"], f["W1"]
    common = {
        "wqt": np.ascontiguousarray(Wq.transpose(1, 0, 2).astype(bf)),
        "wkt": np.ascontiguousarray(Wk.transpose(1, 0, 2).astype(bf)),
        # Wv is [H, E, E]; same layout as Wq with D == E
        "wvt": np.ascontiguousarray(Wv.transpose(1, 0, 2).astype(bf)),
        "bqr": np.ascontiguousarray(f["bq"][None, :, :].astype(bf)),
        "bkr": np.ascontiguousarray(f["bk"][None, :, :].astype(bf)),
        "bvr": np.ascontiguousarray(f["bv"][None, :, :].astype(bf)),
        "bkc": np.ascontiguousarray(f["bk"].transpose(1, 0)),
        "w1t": np.ascontiguousarray(
            W1.reshape(H, 128, E).transpose(1, 0, 2).astype(bf)),
        "w2c": np.ascontiguousarray(f["W2"].astype(bf)),
        "b1c": np.ascontiguousarray(f["b1"][:, None]),
        "b2r": np.ascontiguousarray(f["b2"][None, :].astype(bf)),
    }
    x = f["x"]
    in_maps = []
    for c in range(NCORES):
        m = dict(common)
        xs = np.ascontiguousarray(x[c * BL:(c + 1) * BL])
        m["x"] = xs
        m["xt"] = np.ascontiguousarray(xs.transpose(2, 0, 1).astype(bf))
        in_maps.append(m)
    return in_maps


def kernel(**inputs):
    global _NC_CACHE
    if _NC_CACHE is None:
        _NC_CACHE = _build()
    nc = _NC_CACHE

    in_maps = make_in_maps(inputs)
    res = run_bass_kernel_spmd(nc, in_maps, list(range(NCORES)))
    out = np.concatenate([res.results[c]["out"] for c in range(NCORES)],
                         axis=0).astype(np.float32)
    # final LayerNorm affine (gamma == 1, beta == 0 in practice, but apply
    # faithfully host-side)
    gamma = np.asarray(inputs["gamma"], dtype=np.float32)
    beta = np.asarray(inputs["beta"], dtype=np.float32)
    return out * gamma[None, None, :] + beta[None, None, :]


if __name__ == "__main__":
    nc = _build()
    print("built ok")


# revision 29
# speedup vs baseline: 1.3470x; 1.3470x over previous
"""Trainium2 Bass kernel for nn_AttentionModule (batch-softmax attention + MLP).

Sharding: batch-parallel over the 8 NeuronCores (8 of 64 batches per core).
The softmax over dim=0 (batch) needs the cross-core sum of exp(scores): a
bf16 AllReduce of Z[h, m, n] per half-head (0.25 MB), pipelined against the
other half's score/exp compute.  Everything else is local.

v2 structure (per core, b = local batch 0..7, h = head 0..3):
  proj (token-major, all heads grouped per 128-token chunk):
    PQ|PV     = x_c @ [Wq|Wv] + bias      (512-free matmuls + rank-1 bias)
    PK        = x_c @ Wk + bias           (stats only -- never applied)
    stats     = bn_stats on PSUM (DVE), combined with elementwise ops
    rstd      = exp(-0.5*ln(var+eps))     (ACT, same table set as the
                softmax Exp; K's rstd also folds in the 1/11 score scale)
    Q,V       = (raw - mu) * rstd         (gpsimd, from bf16 SBUF copies)
    qt        = Q^T per head              (PE transpose, deferred one batch)
  attention (head-major, AllReduce per half-head):
    KT_raw    = Wk_h^T @ x_b^T + bk       (feature-major K: NO LayerNorm
                apply and NO transpose -- the LN'd Q is zero-mean along d,
                so K's mean subtraction cancels in the scores, and K's rstd
                is applied as the per-partition Exp scale)
    E^T[m,n]  = exp(rk[m]/11 * (KT_raw chunk)^T @ qt)
    Z-half    = sum_b E (DVE bf16 tree) -> bf16 AllReduce -> R = exp(-ln(Z))
    A^T       += V_c^T @ (E^T * R)        (PSUM accumulation, at += per half)
  mlp (per b): h1^T = relu(W1^T A_cat^T + b1); mlp = relu(h1 W2 + b2);
    out = LN(x + mlp)  (gamma/beta applied host-side; they are affine
    constants of the final LayerNorm)
"""

import sys

for _p in ("/opt/trn_rl_repo", "/opt/pypackages"):
    if _p not in sys.path:
        sys.path.append(_p)

from contextlib import ExitStack

import numpy as np

import concourse.bass as bass
import concourse.tile as tile
from concourse import bacc, masks, mybir
from concourse.bass_utils import run_bass_kernel_spmd

B, N, E, H = 64, 512, 128, 4
NCORES = 8
BL = B // NCORES          # local batches per core
NCH = N // 128            # 128-row chunks per sequence
EPS = 1e-5
NLN11 = -float(np.log(11.0))  # fold the 1/11 score scale into K's rstd

F32 = mybir.dt.float32
BF16 = mybir.dt.bfloat16
AF = mybir.ActivationFunctionType
ALU = mybir.AluOpType


def _reordered_act_tables(arch):
    """Activation tables filtered so one set serves every function we use.

    The table-load placement pass assigns each activation function to the
    first set that contains it, which splits Exp (exp_and_others) and Ln
    (natural_log) into different sets and makes the ACT engine ping-pong
    its spline tables (~1.3us per switch).  This kernel only uses
    {Identity, Copy, Exp, Ln, Relu}, all present in
    natural_log_exp_and_others; removing them from every other set forces
    the pass to pick that one set for all activations.  Set IDs are
    positional (index into act_info.json), so the original dict order is
    preserved.
    """
    from concourse.hw_specs import get_activation_tables

    tabs = get_activation_tables(arch)
    pref = "natural_log_exp_and_others"
    keep = tabs[pref]
    return {name: (fns if name == pref else (fns - keep))
            for name, fns in tabs.items()}


def _build():
    nc = bacc.Bacc(None, target_bir_lowering=False, debug=False)
    _orig_tables = bacc.get_activation_tables
    bacc.get_activation_tables = _reordered_act_tables

    x_ext = nc.declare_dram_parameter("x", [BL, N, E], F32, isOutput=False)
    xt_ext = nc.declare_dram_parameter("xt", [E, BL, N], BF16, isOutput=False)
    wq_ext = nc.declare_dram_parameter("wqt", [E, H, 128], BF16, isOutput=False)
    wk_ext = nc.declare_dram_parameter("wkt", [E, H, 128], BF16, isOutput=False)
    wv_ext = nc.declare_dram_parameter("wvt", [E, H, 128], BF16, isOutput=False)
    bq_ext = nc.declare_dram_parameter("bqr", [1, H, 128], BF16, isOutput=False)
    bk_ext = nc.declare_dram_parameter("bkr", [1, H, 128], BF16, isOutput=False)
    bv_ext = nc.declare_dram_parameter("bvr", [1, H, 128], BF16, isOutput=False)
    bkc_ext = nc.declare_dram_parameter("bkc", [128, H], F32, isOutput=False)
    w1_ext = nc.declare_dram_parameter("w1t", [E, H, 128], BF16, isOutput=False)
    w2_ext = nc.declare_dram_parameter("w2c", [128, 128], BF16, isOutput=False)
    b1_ext = nc.declare_dram_parameter("b1c", [128, 1], F32, isOutput=False)
    b2_ext = nc.declare_dram_parameter("b2r", [1, 128], BF16, isOutput=False)
    out_ext = nc.declare_dram_parameter("out", [BL, N, E], F32, isOutput=True)

    with tile.TileContext(nc) as tc:
        with ExitStack() as ctx:
            _emit(ctx, tc, x_ext, xt_ext, wq_ext, wk_ext, wv_ext, bq_ext,
                  bk_ext, bv_ext, bkc_ext, w1_ext, w2_ext, b1_ext, b2_ext,
                  out_ext)
    try:
        nc.compile()
    finally:
        bacc.get_activation_tables = _orig_tables
    return nc


def _emit(ctx, tc, x_ext, xt_ext, wq_ext, wk_ext, wv_ext, bq_ext, bk_ext,
          bv_ext, bkc_ext, w1_ext, w2_ext, b1_ext, b2_ext, out_ext):
    nc = tc.nc

    persist = ctx.enter_context(tc.tile_pool(name="persist", bufs=1))
    qn_pool = ctx.enter_context(tc.tile_pool(name="qn", bufs=2))
    st_pool = ctx.enter_context(tc.tile_pool(name="st", bufs=2))
    stat = ctx.enter_context(tc.tile_pool(name="stat", bufs=10))
    expd_pool = ctx.enter_context(tc.tile_pool(name="expd", bufs=16))
    ztmp_pool = ctx.enter_context(tc.tile_pool(name="ztmp", bufs=2))
    lnz_pool = ctx.enter_context(tc.tile_pool(name="lnz", bufs=1))
    rr_pool = ctx.enter_context(tc.tile_pool(name="rr", bufs=3))
    ktb_pool = ctx.enter_context(tc.tile_pool(name="ktb", bufs=8))
    mlpp = ctx.enter_context(tc.tile_pool(name="mlpp", bufs=2))
    dram = ctx.enter_context(tc.tile_pool(name="dram", bufs=4, space="DRAM"))

    ps_big = ctx.enter_context(tc.tile_pool(name="ps_big", bufs=2,
                                            space="PSUM"))
    ps_small = ctx.enter_context(tc.tile_pool(name="ps_small", bufs=4,
                                              space="PSUM"))

    # ---- tiny dummy collective: pulls the one-time collective-init
    # barrier into the load phase ----
    dmy_in = dram.tile([128, 4], F32, tag="dmy_in")
    dmy_out = dram.tile([128, 4], F32, tag="dmy_out")
    zt = stat.tile([128, 4], F32, tag="zt")
    nc.vector.memset(zt[:], 0.0)
    nc.sync.dma_start(dmy_in[:], zt[:])
    replica = [list(range(NCORES))]
    nc.gpsimd.collective_compute("AllReduce", ALU.add,
                                 replica_groups=replica,
                                 ins=[dmy_in.opt()], outs=[dmy_out.opt()])

    # ---- constants & weights (all pre-laid-out host-side, direct DMA) ----
    ident = persist.tile([128, 128], BF16)
    masks.make_identity(nc, ident[:])
    ones1 = persist.tile([1, 128], BF16)
    nc.vector.memset(ones1[:], 1.0)
    epst = persist.tile([128, 1], F32)
    nc.vector.memset(epst[:], EPS)
    nln11t = persist.tile([128, 1], F32)
    nc.vector.memset(nln11t[:], NLN11)

    xt = persist.tile([128, BL, N], BF16)
    nc.sync.dma_start(xt[:], xt_ext[:])
    wq_sb = persist.tile([128, H, 128], BF16)
    nc.sync.dma_start(wq_sb[:], wq_ext[:])
    wk_sb = persist.tile([128, H, 128], BF16)
    nc.sync.dma_start(wk_sb[:], wk_ext[:])
    wv_sb = persist.tile([128, H, 128], BF16)
    nc.sync.dma_start(wv_sb[:], wv_ext[:])
    bq_sb = persist.tile([1, H, 128], BF16)
    nc.sync.dma_start(bq_sb[:], bq_ext[:])
    bk_sb = persist.tile([1, H, 128], BF16)
    nc.sync.dma_start(bk_sb[:], bk_ext[:])
    bv_sb = persist.tile([1, H, 128], BF16)
    nc.sync.dma_start(bv_sb[:], bv_ext[:])
    bkc_sb = persist.tile([128, H], F32)
    nc.sync.dma_start(bkc_sb[:], bkc_ext[:])
    w1_sb = persist.tile([128, H, 128], BF16)
    nc.sync.dma_start(w1_sb[:], w1_ext[:])
    w2_sb = persist.tile([128, 128], BF16)
    nc.sync.dma_start(w2_sb[:], w2_ext[:])
    b1col = persist.tile([128, 1], F32)
    nc.sync.dma_start(b1col[:], b1_ext[:])
    b2row = persist.tile([1, 128], BF16)
    nc.sync.dma_start(b2row[:], b2_ext[:])

    # ---- persistent per-core state ----
    qt_sb = persist.tile([128, BL, H, N], BF16)     # Q^T per (b, h)
    vh_sb = persist.tile([128, BL, NCH, H, 128], BF16)  # normalized V
    at_sb = persist.tile([128, BL, H, N], BF16)     # A^T per (b, h)
    rk_sb = persist.tile([128, BL, NCH, H], F32)    # K rstd / 11 columns

    # ================= projection phase =================
    pending_tp = None

    def _transposes(b, qn):
        for h in range(H):
            qtp = ps_small.tile([128, N], BF16, tag="small")
            for c in range(NCH):
                nc.tensor.transpose(qtp[:, c * 128:(c + 1) * 128],
                                    qn[:, h, c, :], ident[:])
            nc.scalar.copy(qt_sb[:, b, h, :], qtp[:])

    for b in range(BL):
        qn = qn_pool.tile([128, H, NCH, 128], BF16, tag="qn", name=f"qn_{b}")
        st = st_pool.tile([128, NCH, 3, H, 6], F32, tag="st", name=f"st_{b}")
        for c in range(NCH):
            xt_c = xt[:, b, c * 128:(c + 1) * 128]
            ppqv = ps_big.tile([128, 2, H, 128], F32, tag="big")
            nc.tensor.matmul(ppqv[:, 0], xt_c, wq_sb[:], start=True,
                             stop=False)
            nc.tensor.matmul(ppqv[:, 0], ones1[:], bq_sb[:], start=False,
                             stop=True)
            nc.tensor.matmul(ppqv[:, 1], xt_c, wv_sb[:], start=True,
                             stop=False)
            nc.tensor.matmul(ppqv[:, 1], ones1[:], bv_sb[:], start=False,
                             stop=True)
            ppk = ps_small.tile([128, H, 128], F32, tag="small")
            nc.tensor.matmul(ppk[:], xt_c, wk_sb[:], start=True, stop=False)
            nc.tensor.matmul(ppk[:], ones1[:], bk_sb[:], start=False,
                             stop=True)
            # st slots: 0 = q, 1 = v, 2 = k
            for hh in range(H):
                nc.vector.bn_stats(st[:, c, 0, hh, :], ppqv[:, 0, hh, :])
                nc.vector.bn_stats(st[:, c, 1, hh, :], ppqv[:, 1, hh, :])
                nc.vector.bn_stats(st[:, c, 2, hh, :], ppk[:, hh, :])

            # per-chunk Q/V mean/var -> rstd (frees the PSUM slot quickly)
            sqv = st[:, c, 0:2, :, :]
            muc = stat.tile([128, 2, H], F32, tag="muc", name=f"mu_{b}_{c}")
            nc.vector.tensor_add(muc[:], sqv[:, :, :, 1], sqv[:, :, :, 4])
            nc.vector.tensor_scalar(muc[:], muc[:], 0.5, None, op0=ALU.mult)
            dhc = stat.tile([128, 2, H], F32, tag="dhc", name=f"dh_{b}_{c}")
            nc.vector.tensor_sub(dhc[:], sqv[:, :, :, 1], sqv[:, :, :, 4])
            nc.vector.tensor_scalar(dhc[:], dhc[:], 0.5, None, op0=ALU.mult)
            nc.vector.tensor_mul(dhc[:], dhc[:], dhc[:])
            varc = stat.tile([128, 2, H], F32, tag="varc", name=f"var_{b}_{c}")
            nc.vector.tensor_add(varc[:], sqv[:, :, :, 2], sqv[:, :, :, 5])
            nc.vector.scalar_tensor_tensor(varc[:], varc[:], 1.0 / 128.0,
                                           dhc[:], op0=ALU.mult, op1=ALU.add)
            lnc = stat.tile([128, 2, H], F32, tag="lnc", name=f"ln_{b}_{c}")
            nc.scalar.activation(lnc[:], varc[:], AF.Ln, bias=epst[:])
            rstdc = stat.tile([128, 2, H], F32, tag="rstdc",
                              name=f"rstd_{b}_{c}")
            nc.scalar.activation(rstdc[:], lnc[:], AF.Exp, scale=-0.5)
            nmqc = stat.tile([128, H], F32, tag="nmqc", name=f"nmq_{b}_{c}")
            nc.vector.scalar_tensor_tensor(nmqc[:], muc[:, 0, :], -1.0,
                                           rstdc[:, 0, :], op0=ALU.mult,
                                           op1=ALU.mult)
            # applies double as the PSUM->SBUF moves: Q on ACT, V on DVE
            for hh in range(H):
                nc.scalar.activation(qn[:, hh, c, :], ppqv[:, 0, hh, :],
                                     AF.Identity, bias=nmqc[:, hh:hh + 1],
                                     scale=rstdc[:, 0, hh:hh + 1])
                nc.vector.tensor_scalar(vh_sb[:, b, c, hh, :],
                                        ppqv[:, 1, hh, :],
                                        muc[:, 1, hh:hh + 1],
                                        rstdc[:, 1, hh:hh + 1],
                                        op0=ALU.subtract, op1=ALU.mult)

        if pending_tp is not None:
            pending_tp()
            pending_tp = None

        # K rstd (batched per b; the k stats live in SBUF, no PSUM held)
        sk = st[:, :, 2, :, :]
        muk = stat.tile([128, NCH, H], F32, tag="muk", name=f"muk_{b}")
        nc.vector.tensor_sub(muk[:], sk[:, :, :, 1], sk[:, :, :, 4])
        # muk now holds (mu_e - mu_o); halve and square for the var term
        nc.vector.tensor_scalar(muk[:], muk[:], 0.5, None, op0=ALU.mult)
        nc.vector.tensor_mul(muk[:], muk[:], muk[:])
        vark = stat.tile([128, NCH, H], F32, tag="vark", name=f"vark_{b}")
        nc.vector.tensor_add(vark[:], sk[:, :, :, 2], sk[:, :, :, 5])
        nc.vector.scalar_tensor_tensor(vark[:], vark[:], 1.0 / 128.0, muk[:],
                                       op0=ALU.mult, op1=ALU.add)
        lnk = stat.tile([128, NCH, H], F32, tag="lnk", name=f"lnk_{b}")
        nc.scalar.activation(lnk[:], vark[:], AF.Ln, bias=epst[:])
        nc.scalar.activation(rk_sb[:, b, :, :], lnk[:], AF.Exp,
                             scale=-0.5, bias=nln11t[:])

        pending_tp = (lambda b=b, qn=qn: _transposes(b, qn))

    pending_tp()

    # ================= attention phase =================
    def _ktb(h):
        tiles = []
        for b in range(BL):
            ktp = ps_small.tile([128, N], F32, tag="small")
            nc.tensor.matmul(ktp[:], wk_sb[:, h, :], xt[:, b, :],
                             start=True, stop=True)
            kt = ktb_pool.tile([128, N], BF16, tag="ktb",
                               name=f"ktb_{h}_{b}")
            nc.scalar.activation(kt[:], ktp[:], AF.Identity,
                                 bias=bkc_sb[:, h:h + 1])
            tiles.append(kt)
        return tiles

    def _half(h, half, ktb):
        """scores + exp for one (head, m-half); returns exp tiles + launches AR."""
        eds = []
        for b in range(BL):
            ed = expd_pool.tile([128, 2, N], BF16, tag="expd",
                                name=f"ed_{h}_{half}_{b}")
            for ci in range(2):
                c = half * 2 + ci
                ss = ps_big.tile([128, N], F32, tag="big")
                nc.tensor.matmul(ss[:], ktb[b][:, c * 128:(c + 1) * 128],
                                 qt_sb[:, b, h, :], start=True, stop=True)
                nc.scalar.activation(ed[:, ci, :], ss[:], AF.Exp,
                                     scale=rk_sb[:, b, c, h:h + 1])
            eds.append(ed)
        # Z = sum_b exp : accumulated by the DMA engines (CCE add) on the
        # way into the AllReduce input buffer; costs no ACT/DVE time
        zin = dram.tile([128, 2, N], BF16, tag="zin", name=f"zin_{h}_{half}")
        zout = dram.tile([128, 2, N], BF16, tag="zout",
                         name=f"zout_{h}_{half}")
        for i in range(BL):
            nc.gpsimd.dma_start(zin[:], eds[i][:],
                                accum_op=(ALU.bypass if i == 0 else ALU.add))
        nc.gpsimd.collective_compute("AllReduce", ALU.add,
                                     replica_groups=replica,
                                     ins=[zin.opt()], outs=[zout.opt()])
        return eds, zout

    def _rr(h, half, zout):
        zg = ztmp_pool.tile([128, 2, N], BF16, tag="zg",
                            name=f"zg_{h}_{half}")
        nc.sync.dma_start(zg[:], zout[:])
        rr = rr_pool.tile([128, 2, N], BF16, tag="rr", name=f"rr_{h}_{half}")
        for ci in range(2):
            lnz = lnz_pool.tile([128, N], F32, tag="lnz",
                                name=f"lnz_{h}_{half}_{ci}")
            nc.scalar.activation(lnz[:], zg[:, ci, :], AF.Ln)
            nc.scalar.activation(rr[:, ci, :], lnz[:], AF.Exp, scale=-1.0)
        return rr

    def _pv(h, half, eds, rr, and_mlp=False):
        for b in range(BL):
            ed = eds[b]
            pa = ps_small.tile([128, N], F32, tag="small")
            for ci in range(2):
                c = half * 2 + ci
                nc.gpsimd.tensor_mul(ed[:, ci, :], ed[:, ci, :],
                                     rr[:, ci, :])
                nc.tensor.matmul(pa[:], vh_sb[:, b, c, h, :], ed[:, ci, :],
                                 start=(ci == 0), stop=(ci == 1))
            if half == 0:
                nc.vector.tensor_copy(at_sb[:, b, h, :], pa[:])
            else:
                nc.vector.tensor_add(at_sb[:, b, h, :], at_sb[:, b, h, :],
                                     pa[:])
            if and_mlp:
                _mlp_b(b)

    def _mlp_b(b):
        p1 = ps_big.tile([128, N], F32, tag="big", name=f"p1_{b}")
        for hh in range(H):
            nc.tensor.matmul(p1[:], w1_sb[:, hh, :], at_sb[:, b, hh, :],
                             start=(hh == 0), stop=(hh == H - 1))
        h1t = mlpp.tile([128, N], BF16, tag="h1t", name=f"h1t_{b}")
        nc.scalar.activation(h1t[:], p1[:], AF.Relu, bias=b1col[:])
        xres = mlpp.tile([128, NCH, 128], F32, tag="xres", name=f"xres_{b}")
        nc.sync.dma_start(xres[:],
                          x_ext[b].rearrange("(c p) e -> p c e", p=128))
        p2 = ps_small.tile([128, NCH, 128], F32, tag="small", name=f"p2_{b}")
        for c in range(NCH):
            nc.tensor.matmul(p2[:, c, :], h1t[:, c * 128:(c + 1) * 128],
                             w2_sb[:], start=True, stop=False)
            nc.tensor.matmul(p2[:, c, :], ones1[:], b2row[:], start=False,
                             stop=True)
        ys = mlpp.tile([128, NCH, 128], BF16, tag="ys", name=f"ys_{b}")
        nc.scalar.activation(ys[:], p2[:], AF.Relu)
        ysum = mlpp.tile([128, NCH, 128], BF16, tag="ysum", name=f"ysum_{b}")
        nc.gpsimd.tensor_add(ysum[:], ys[:], xres[:])
        st8 = stat.tile([128, NCH, 6], F32, tag="st8", name=f"st8_{b}")
        for c in range(NCH):
            nc.vector.bn_stats(st8[:, c, :], ysum[:, c, :])
        mu8 = stat.tile([128, NCH], F32, tag="mu8", name=f"mu8_{b}")
        nc.vector.tensor_add(mu8[:], st8[:, :, 1], st8[:, :, 4])
        nc.vector.tensor_scalar(mu8[:], mu8[:], 0.5, None, op0=ALU.mult)
        dh8 = stat.tile([128, NCH], F32, tag="dh8", name=f"dh8_{b}")
        nc.vector.tensor_sub(dh8[:], st8[:, :, 1], st8[:, :, 4])
        nc.vector.tensor_scalar(dh8[:], dh8[:], 0.5, None, op0=ALU.mult)
        nc.vector.tensor_mul(dh8[:], dh8[:], dh8[:])
        var8 = stat.tile([128, NCH], F32, tag="var8", name=f"var8_{b}")
        nc.vector.tensor_add(var8[:], st8[:, :, 2], st8[:, :, 5])
        nc.vector.scalar_tensor_tensor(var8[:], var8[:], 1.0 / 128.0, dh8[:],
                                       op0=ALU.mult, op1=ALU.add)
        ln8 = stat.tile([128, NCH], F32, tag="ln8", name=f"ln8_{b}")
        nc.scalar.activation(ln8[:], var8[:], AF.Ln, bias=epst[:])
        rstd8 = stat.tile([128, NCH], F32, tag="rstd8", name=f"rstd8_{b}")
        nc.scalar.activation(rstd8[:], ln8[:], AF.Exp, scale=-0.5)
        yo = mlpp.tile([128, NCH, 128], F32, tag="yo", name=f"yo_{b}")
        for c in range(NCH):
            nc.gpsimd.tensor_scalar(yo[:, c, :], ysum[:, c, :],
                                    mu8[:, c:c + 1], rstd8[:, c:c + 1],
                                    op0=ALU.subtract, op1=ALU.mult)
        nc.sync.dma_start(out_ext[b].rearrange("(c p) e -> p c e", p=128),
                          yo[:])

    ktb = _ktb(0)
    for h in range(H):
        eds0, zout0 = _half(h, 0, ktb)
        eds1, zout1 = _half(h, 1, ktb)
        rr0 = _rr(h, 0, zout0)
        _pv(h, 0, eds0, rr0)
        if h < H - 1:
            ktb = _ktb(h + 1)
        rr1 = _rr(h, 1, zout1)
        _pv(h, 1, eds1, rr1, and_mlp=(h == H - 1))


_NC_CACHE = None


def make_in_maps(inputs):
    import ml_dtypes

    bf = ml_dtypes.bfloat16
    f = {k: np.asarray(v, dtype=np.float32) for k, v in inputs.items()}
    Wq, Wk, Wv, W1 = f["Wq"], f["Wk"], f["Wv"], f["W1"]
    common = {
        "wqt": np.ascontiguousarray(Wq.transpose(1, 0, 2).astype(bf)),
        "wkt": np.ascontiguousarray(Wk.transpose(1, 0, 2).astype(bf)),
        # Wv is [H, E, E]; same layout as Wq with D == E
        "wvt": np.ascontiguousarray(Wv.transpose(1, 0, 2).astype(bf)),
        "bqr": np.ascontiguousarray(f["bq"][None, :, :].astype(bf)),
        "bkr": np.ascontiguousarray(f["bk"][None, :, :].astype(bf)),
        "bvr": np.ascontiguousarray(f["bv"][None, :, :].astype(bf)),
        "bkc": np.ascontiguousarray(f["bk"].transpose(1, 0)),
        "w1t": np.ascontiguousarray(
            W1.reshape(H, 128, E).transpose(1, 0, 2).astype(bf)),
        "w2c": np.ascontiguousarray(f["W2"].astype(bf)),
        "b1c": np.ascontiguousarray(f["b1"][:, None]),
        "b2r": np.ascontiguousarray(f["b2"][None, :].astype(bf)),
    }
    x = f["x"]
    in_maps = []
    for c in range(NCORES):
        m = dict(common)
        xs = np.ascontiguousarray(x[c * BL:(c + 1) * BL])
        m["x"] = xs
        m["xt"] = np.ascontiguousarray(xs.transpose(2, 0, 1).astype(bf))
        in_maps.append(m)
    return in_maps


def kernel(**inputs):
    global _NC_CACHE
    if _NC_CACHE is None:
        _NC_CACHE = _build()
    nc = _NC_CACHE

    in_maps = make_in_maps(inputs)
    res = run_bass_kernel_spmd(nc, in_maps, list(range(NCORES)))
    out = np.concatenate([res.results[c]["out"] for c in range(NCORES)],
                         axis=0).astype(np.float32)
    # final LayerNorm affine (gamma == 1, beta == 0 in practice, but apply
    # faithfully host-side)
    gamma = np.asarray(inputs["gamma"], dtype=np.float32)
    beta = np.asarray(inputs["beta"], dtype=np.float32)
    return out * gamma[None, None, :] + beta[None, None, :]


if __name__ == "__main__":
    nc = _build()
    print("built ok")


# revision 30
# speedup vs baseline: 1.9678x; 1.4608x over previous
"""Trainium2 Bass kernel for nn_AttentionModule (batch-softmax attention + MLP).

Sharding: batch-parallel over the 8 NeuronCores (8 of 64 batches per core).
The softmax over dim=0 (batch) needs the cross-core sum of exp(scores): a
bf16 AllReduce of Z[h, m, n] per half-head (0.25 MB), pipelined against the
other half's score/exp compute.  Everything else is local.

v2 structure (per core, b = local batch 0..7, h = head 0..3):
  proj (token-major, all heads grouped per 128-token chunk):
    PQ|PV     = x_c @ [Wq|Wv] + bias      (512-free matmuls + rank-1 bias)
    PK        = x_c @ Wk + bias           (stats only -- never applied)
    stats     = bn_stats on PSUM (DVE), combined with elementwise ops
    rstd      = exp(-0.5*ln(var+eps))     (ACT, same table set as the
                softmax Exp; K's rstd also folds in the 1/11 score scale)
    Q,V       = (raw - mu) * rstd         (gpsimd, from bf16 SBUF copies)
    qt        = Q^T per head              (PE transpose, deferred one batch)
  attention (head-major, AllReduce per half-head):
    KT_raw    = Wk_h^T @ x_b^T + bk       (feature-major K: NO LayerNorm
                apply and NO transpose -- the LN'd Q is zero-mean along d,
                so K's mean subtraction cancels in the scores, and K's rstd
                is applied as the per-partition Exp scale)
    E^T[m,n]  = exp(rk[m]/11 * (KT_raw chunk)^T @ qt)
    Z-half    = sum_b E (DVE bf16 tree) -> bf16 AllReduce -> R = exp(-ln(Z))
    A^T       += V_c^T @ (E^T * R)        (PSUM accumulation, at += per half)
  mlp (per b): h1^T = relu(W1^T A_cat^T + b1); mlp = relu(h1 W2 + b2);
    out = LN(x + mlp)  (gamma/beta applied host-side; they are affine
    constants of the final LayerNorm)
"""

import sys

for _p in ("/opt/trn_rl_repo", "/opt/pypackages"):
    if _p not in sys.path:
        sys.path.append(_p)

from contextlib import ExitStack

import numpy as np

import concourse.bass as bass
import concourse.tile as tile
from concourse import bacc, masks, mybir
from concourse.bass_utils import run_bass_kernel_spmd

B, N, E, H = 64, 512, 128, 4
NCORES = 8
BL = B // NCORES          # local batches per core
NCH = N // 128            # 128-row chunks per sequence
EPS = 1e-5
NLN11 = -float(np.log(11.0))  # fold the 1/11 score scale into K's rstd

F32 = mybir.dt.float32
BF16 = mybir.dt.bfloat16
AF = mybir.ActivationFunctionType
ALU = mybir.AluOpType


def _reordered_act_tables(arch):
    """Activation tables filtered so one set serves every function we use.

    The table-load placement pass assigns each activation function to the
    first set that contains it, which splits Exp (exp_and_others) and Ln
    (natural_log) into different sets and makes the ACT engine ping-pong
    its spline tables (~1.3us per switch).  This kernel only uses
    {Identity, Copy, Exp, Ln, Relu}, all present in
    natural_log_exp_and_others; removing them from every other set forces
    the pass to pick that one set for all activations.  Set IDs are
    positional (index into act_info.json), so the original dict order is
    preserved.
    """
    from concourse.hw_specs import get_activation_tables

    tabs = get_activation_tables(arch)
    pref = "natural_log_exp_and_others"
    keep = tabs[pref]
    return {name: (fns if name == pref else (fns - keep))
            for name, fns in tabs.items()}


def _build():
    nc = bacc.Bacc(None, target_bir_lowering=False, debug=False)
    _orig_tables = bacc.get_activation_tables
    bacc.get_activation_tables = _reordered_act_tables

    x_ext = nc.declare_dram_parameter("x", [BL, N, E], F32, isOutput=False)
    xt_ext = nc.declare_dram_parameter("xt", [E, BL, N], BF16, isOutput=False)
    wq_ext = nc.declare_dram_parameter("wqt", [E, H, 128], BF16, isOutput=False)
    wk_ext = nc.declare_dram_parameter("wkt", [E, H, 128], BF16, isOutput=False)
    wv_ext = nc.declare_dram_parameter("wvt", [E, H, 128], BF16, isOutput=False)
    bq_ext = nc.declare_dram_parameter("bqr", [1, H, 128], BF16, isOutput=False)
    bk_ext = nc.declare_dram_parameter("bkr", [1, H, 128], BF16, isOutput=False)
    bv_ext = nc.declare_dram_parameter("bvr", [1, H, 128], BF16, isOutput=False)
    bkc_ext = nc.declare_dram_parameter("bkc", [128, H], F32, isOutput=False)
    w1_ext = nc.declare_dram_parameter("w1t", [E, H, 128], BF16, isOutput=False)
    w2_ext = nc.declare_dram_parameter("w2c", [128, 128], BF16, isOutput=False)
    b1_ext = nc.declare_dram_parameter("b1c", [128, 1], F32, isOutput=False)
    b2_ext = nc.declare_dram_parameter("b2r", [1, 128], BF16, isOutput=False)
    out_ext = nc.declare_dram_parameter("out", [BL, N, E], F32, isOutput=True)

    with tile.TileContext(nc) as tc:
        with ExitStack() as ctx:
            _emit(ctx, tc, x_ext, xt_ext, wq_ext, wk_ext, wv_ext, bq_ext,
                  bk_ext, bv_ext, bkc_ext, w1_ext, w2_ext, b1_ext, b2_ext,
                  out_ext)
    try:
        nc.compile()
    finally:
        bacc.get_activation_tables = _orig_tables
    return nc


def _emit(ctx, tc, x_ext, xt_ext, wq_ext, wk_ext, wv_ext, bq_ext, bk_ext,
          bv_ext, bkc_ext, w1_ext, w2_ext, b1_ext, b2_ext, out_ext):
    nc = tc.nc

    persist = ctx.enter_context(tc.tile_pool(name="persist", bufs=1))
    qn_pool = ctx.enter_context(tc.tile_pool(name="qn", bufs=2))
    st_pool = ctx.enter_context(tc.tile_pool(name="st", bufs=2))
    stat = ctx.enter_context(tc.tile_pool(name="stat", bufs=10))
    expd_pool = ctx.enter_context(tc.tile_pool(name="expd", bufs=16))
    ztmp_pool = ctx.enter_context(tc.tile_pool(name="ztmp", bufs=2))
    lnz_pool = ctx.enter_context(tc.tile_pool(name="lnz", bufs=1))
    rr_pool = ctx.enter_context(tc.tile_pool(name="rr", bufs=3))
    ktb_pool = ctx.enter_context(tc.tile_pool(name="ktb", bufs=8))
    mlpp = ctx.enter_context(tc.tile_pool(name="mlpp", bufs=2))
    dram = ctx.enter_context(tc.tile_pool(name="dram", bufs=4, space="DRAM"))

    ps_big = ctx.enter_context(tc.tile_pool(name="ps_big", bufs=2,
                                            space="PSUM"))
    ps_small = ctx.enter_context(tc.tile_pool(name="ps_small", bufs=4,
                                              space="PSUM"))

    # ---- tiny dummy collective: pulls the one-time collective-init
    # barrier into the load phase ----
    dmy_in = dram.tile([128, 4], F32, tag="dmy_in")
    dmy_out = dram.tile([128, 4], F32, tag="dmy_out")
    zt = stat.tile([128, 4], F32, tag="zt")
    nc.vector.memset(zt[:], 0.0)
    nc.sync.dma_start(dmy_in[:], zt[:])
    replica = [list(range(NCORES))]
    nc.gpsimd.collective_compute("AllReduce", ALU.add,
                                 replica_groups=replica,
                                 ins=[dmy_in.opt()], outs=[dmy_out.opt()])

    # ---- constants & weights (all pre-laid-out host-side, direct DMA) ----
    ident = persist.tile([128, 128], BF16)
    masks.make_identity(nc, ident[:])
    ones1 = persist.tile([1, 128], BF16)
    nc.vector.memset(ones1[:], 1.0)
    epst = persist.tile([128, 1], F32)
    nc.vector.memset(epst[:], EPS)
    nln11t = persist.tile([128, 1], F32)
    nc.vector.memset(nln11t[:], NLN11)

    xt = persist.tile([128, BL, N], BF16)
    nc.sync.dma_start(xt[:], xt_ext[:])
    wq_sb = persist.tile([128, H, 128], BF16)
    nc.sync.dma_start(wq_sb[:], wq_ext[:])
    wk_sb = persist.tile([128, H, 128], BF16)
    nc.sync.dma_start(wk_sb[:], wk_ext[:])
    wv_sb = persist.tile([128, H, 128], BF16)
    nc.sync.dma_start(wv_sb[:], wv_ext[:])
    bq_sb = persist.tile([1, H, 128], BF16)
    nc.sync.dma_start(bq_sb[:], bq_ext[:])
    bk_sb = persist.tile([1, H, 128], BF16)
    nc.sync.dma_start(bk_sb[:], bk_ext[:])
    bv_sb = persist.tile([1, H, 128], BF16)
    nc.sync.dma_start(bv_sb[:], bv_ext[:])
    bkc_sb = persist.tile([128, H], F32)
    nc.sync.dma_start(bkc_sb[:], bkc_ext[:])
    w1_sb = persist.tile([128, H, 128], BF16)
    nc.sync.dma_start(w1_sb[:], w1_ext[:])
    w2_sb = persist.tile([128, 128], BF16)
    nc.sync.dma_start(w2_sb[:], w2_ext[:])
    b1col = persist.tile([128, 1], F32)
    nc.sync.dma_start(b1col[:], b1_ext[:])
    b2row = persist.tile([1, 128], BF16)
    nc.sync.dma_start(b2row[:], b2_ext[:])

    # ---- persistent per-core state ----
    qt_sb = persist.tile([128, BL, H, N], BF16)     # Q^T per (b, h)
    vh_sb = persist.tile([128, BL, NCH, H, 128], BF16)  # normalized V
    at_sb = persist.tile([128, BL, H, N], BF16)     # A^T per (b, h)
    rk_sb = persist.tile([128, BL, NCH, H], F32)    # K rstd / 11 columns

    # ================= projection phase =================
    pending_tp = None

    def _transposes(b, qn):
        for h in range(H):
            qtp = ps_small.tile([128, N], BF16, tag="small")
            for c in range(NCH):
                nc.tensor.transpose(qtp[:, c * 128:(c + 1) * 128],
                                    qn[:, h, c, :], ident[:])
            nc.scalar.copy(qt_sb[:, b, h, :], qtp[:])

    for b in range(BL):
        qn = qn_pool.tile([128, H, NCH, 128], BF16, tag="qn", name=f"qn_{b}")
        st = st_pool.tile([128, NCH, 3, H, 6], F32, tag="st", name=f"st_{b}")
        for c in range(NCH):
            xt_c = xt[:, b, c * 128:(c + 1) * 128]
            ppqv = ps_big.tile([128, 2, H, 128], F32, tag="big")
            nc.tensor.matmul(ppqv[:, 0], xt_c, wq_sb[:], start=True,
                             stop=False)
            nc.tensor.matmul(ppqv[:, 0], ones1[:], bq_sb[:], start=False,
                             stop=True)
            nc.tensor.matmul(ppqv[:, 1], xt_c, wv_sb[:], start=True,
                             stop=False)
            nc.tensor.matmul(ppqv[:, 1], ones1[:], bv_sb[:], start=False,
                             stop=True)
            ppk = ps_small.tile([128, H, 128], F32, tag="small")
            nc.tensor.matmul(ppk[:], xt_c, wk_sb[:], start=True, stop=False)
            nc.tensor.matmul(ppk[:], ones1[:], bk_sb[:], start=False,
                             stop=True)
            # st slots: 0 = q, 1 = v, 2 = k
            for hh in range(H):
                nc.vector.bn_stats(st[:, c, 0, hh, :], ppqv[:, 0, hh, :])
                nc.vector.bn_stats(st[:, c, 1, hh, :], ppqv[:, 1, hh, :])
                nc.vector.bn_stats(st[:, c, 2, hh, :], ppk[:, hh, :])

            # per-chunk Q/V mean/var -> rstd (frees the PSUM slot quickly)
            sqv = st[:, c, 0:2, :, :]
            muc = stat.tile([128, 2, H], F32, tag="muc", name=f"mu_{b}_{c}")
            nc.vector.tensor_add(muc[:], sqv[:, :, :, 1], sqv[:, :, :, 4])
            nc.vector.tensor_scalar(muc[:], muc[:], 0.5, None, op0=ALU.mult)
            dhc = stat.tile([128, 2, H], F32, tag="dhc", name=f"dh_{b}_{c}")
            nc.vector.tensor_sub(dhc[:], sqv[:, :, :, 1], sqv[:, :, :, 4])
            nc.vector.tensor_scalar(dhc[:], dhc[:], 0.5, None, op0=ALU.mult)
            nc.vector.tensor_mul(dhc[:], dhc[:], dhc[:])
            varc = stat.tile([128, 2, H], F32, tag="varc", name=f"var_{b}_{c}")
            nc.vector.tensor_add(varc[:], sqv[:, :, :, 2], sqv[:, :, :, 5])
            nc.vector.scalar_tensor_tensor(varc[:], varc[:], 1.0 / 128.0,
                                           dhc[:], op0=ALU.mult, op1=ALU.add)
            lnc = stat.tile([128, 2, H], F32, tag="lnc", name=f"ln_{b}_{c}")
            nc.scalar.activation(lnc[:], varc[:], AF.Ln, bias=epst[:])
            rstdc = stat.tile([128, 2, H], F32, tag="rstdc",
                              name=f"rstd_{b}_{c}")
            nc.scalar.activation(rstdc[:], lnc[:], AF.Exp, scale=-0.5)
            nmqc = stat.tile([128, 2, H], F32, tag="nmqc",
                             name=f"nmq_{b}_{c}")
            nc.vector.scalar_tensor_tensor(nmqc[:], muc[:], -1.0,
                                           rstdc[:], op0=ALU.mult,
                                           op1=ALU.mult)
            # applies double as the PSUM->SBUF moves: Q on ACT, V on DVE
            for hh in range(H):
                nc.scalar.activation(qn[:, hh, c, :], ppqv[:, 0, hh, :],
                                     AF.Identity,
                                     bias=nmqc[:, 0, hh:hh + 1],
                                     scale=rstdc[:, 0, hh:hh + 1])
                nc.scalar.activation(vh_sb[:, b, c, hh, :],
                                     ppqv[:, 1, hh, :], AF.Identity,
                                     bias=nmqc[:, 1, hh:hh + 1],
                                     scale=rstdc[:, 1, hh:hh + 1])

        if pending_tp is not None:
            pending_tp()
            pending_tp = None

        # K rstd (batched per b; the k stats live in SBUF, no PSUM held)
        sk = st[:, :, 2, :, :]
        muk = stat.tile([128, NCH, H], F32, tag="muk", name=f"muk_{b}")
        nc.vector.tensor_sub(muk[:], sk[:, :, :, 1], sk[:, :, :, 4])
        # muk now holds (mu_e - mu_o); halve and square for the var term
        nc.vector.tensor_scalar(muk[:], muk[:], 0.5, None, op0=ALU.mult)
        nc.vector.tensor_mul(muk[:], muk[:], muk[:])
        vark = stat.tile([128, NCH, H], F32, tag="vark", name=f"vark_{b}")
        nc.vector.tensor_add(vark[:], sk[:, :, :, 2], sk[:, :, :, 5])
        nc.vector.scalar_tensor_tensor(vark[:], vark[:], 1.0 / 128.0, muk[:],
                                       op0=ALU.mult, op1=ALU.add)
        lnk = stat.tile([128, NCH, H], F32, tag="lnk", name=f"lnk_{b}")
        nc.scalar.activation(lnk[:], vark[:], AF.Ln, bias=epst[:])
        nc.scalar.activation(rk_sb[:, b, :, :], lnk[:], AF.Exp,
                             scale=-0.5, bias=nln11t[:])

        pending_tp = (lambda b=b, qn=qn: _transposes(b, qn))

    pending_tp()

    # ================= attention phase =================
    def _ktb(h):
        tiles = []
        for b in range(BL):
            ktp = ps_small.tile([128, N], F32, tag="small")
            nc.tensor.matmul(ktp[:], wk_sb[:, h, :], xt[:, b, :],
                             start=True, stop=True)
            kt = ktb_pool.tile([128, N], BF16, tag="ktb",
                               name=f"ktb_{h}_{b}")
            nc.scalar.activation(kt[:], ktp[:], AF.Identity,
                                 bias=bkc_sb[:, h:h + 1])
            tiles.append(kt)
        return tiles

    def _half(h, half, ktb):
        """scores + exp for one (head, m-half); returns exp tiles + launches AR."""
        eds = []
        for b in range(BL):
            ed = expd_pool.tile([128, 2, N], BF16, tag="expd",
                                name=f"ed_{h}_{half}_{b}")
            for ci in range(2):
                c = half * 2 + ci
                ss = ps_big.tile([128, N], F32, tag="big")
                nc.tensor.matmul(ss[:], ktb[b][:, c * 128:(c + 1) * 128],
                                 qt_sb[:, b, h, :], start=True, stop=True)
                nc.scalar.activation(ed[:, ci, :], ss[:], AF.Exp,
                                     scale=rk_sb[:, b, c, h:h + 1])
            eds.append(ed)
        # Z = sum_b exp : serial bf16 accumulation on DVE (idle in this phase)
        zs = ztmp_pool.tile([128, 2, N], BF16, tag="zsum",
                            name=f"zs_{h}_{half}")
        nc.vector.tensor_add(zs[:], eds[0][:], eds[1][:])
        for i in range(2, BL):
            nc.vector.tensor_add(zs[:], zs[:], eds[i][:])
        zin = dram.tile([128, 2, N], BF16, tag="zin", name=f"zin_{h}_{half}")
        zout = dram.tile([128, 2, N], BF16, tag="zout",
                         name=f"zout_{h}_{half}")
        nc.sync.dma_start(zin[:], zs[:])
        nc.gpsimd.collective_compute("AllReduce", ALU.add,
                                     replica_groups=replica,
                                     ins=[zin.opt()], outs=[zout.opt()])
        return eds, zout

    def _rr(h, half, zout):
        zg = ztmp_pool.tile([128, 2, N], BF16, tag="zg",
                            name=f"zg_{h}_{half}")
        nc.sync.dma_start(zg[:], zout[:])
        rr = rr_pool.tile([128, 2, N], BF16, tag="rr", name=f"rr_{h}_{half}")
        for ci in range(2):
            lnz = lnz_pool.tile([128, N], F32, tag="lnz",
                                name=f"lnz_{h}_{half}_{ci}")
            nc.scalar.activation(lnz[:], zg[:, ci, :], AF.Ln)
            nc.scalar.activation(rr[:, ci, :], lnz[:], AF.Exp, scale=-1.0)
        return rr

    def _pv(h, half, eds, rr, and_mlp=False):
        for b in range(BL):
            ed = eds[b]
            pa = ps_small.tile([128, N], F32, tag="small")
            for ci in range(2):
                c = half * 2 + ci
                nc.vector.tensor_mul(ed[:, ci, :], ed[:, ci, :],
                                     rr[:, ci, :])
                nc.tensor.matmul(pa[:], vh_sb[:, b, c, h, :], ed[:, ci, :],
                                 start=(ci == 0), stop=(ci == 1))
            if half == 0:
                nc.vector.tensor_copy(at_sb[:, b, h, :], pa[:])
            else:
                nc.vector.tensor_add(at_sb[:, b, h, :], at_sb[:, b, h, :],
                                     pa[:])
            if and_mlp:
                _mlp_b(b)

    def _mlp_b(b):
        p1 = ps_big.tile([128, N], F32, tag="big", name=f"p1_{b}")
        for hh in range(H):
            nc.tensor.matmul(p1[:], w1_sb[:, hh, :], at_sb[:, b, hh, :],
                             start=(hh == 0), stop=(hh == H - 1))
        h1t = mlpp.tile([128, N], BF16, tag="h1t", name=f"h1t_{b}")
        nc.scalar.activation(h1t[:], p1[:], AF.Relu, bias=b1col[:])
        xres = mlpp.tile([128, NCH, 128], F32, tag="xres", name=f"xres_{b}")
        nc.sync.dma_start(xres[:],
                          x_ext[b].rearrange("(c p) e -> p c e", p=128))
        p2 = ps_small.tile([128, NCH, 128], F32, tag="small", name=f"p2_{b}")
        for c in range(NCH):
            nc.tensor.matmul(p2[:, c, :], h1t[:, c * 128:(c + 1) * 128],
                             w2_sb[:], start=True, stop=False)
            nc.tensor.matmul(p2[:, c, :], ones1[:], b2row[:], start=False,
                             stop=True)
        ys = mlpp.tile([128, NCH, 128], BF16, tag="ys", name=f"ys_{b}")
        nc.scalar.activation(ys[:], p2[:], AF.Relu)
        ysum = mlpp.tile([128, NCH, 128], BF16, tag="ysum", name=f"ysum_{b}")
        nc.vector.tensor_add(ysum[:], ys[:], xres[:])
        st8 = stat.tile([128, NCH, 6], F32, tag="st8", name=f"st8_{b}")
        for c in range(NCH):
            nc.vector.bn_stats(st8[:, c, :], ysum[:, c, :])
        mu8 = stat.tile([128, NCH], F32, tag="mu8", name=f"mu8_{b}")
        nc.vector.tensor_add(mu8[:], st8[:, :, 1], st8[:, :, 4])
        nc.vector.tensor_scalar(mu8[:], mu8[:], 0.5, None, op0=ALU.mult)
        dh8 = stat.tile([128, NCH], F32, tag="dh8", name=f"dh8_{b}")
        nc.vector.tensor_sub(dh8[:], st8[:, :, 1], st8[:, :, 4])
        nc.vector.tensor_scalar(dh8[:], dh8[:], 0.5, None, op0=ALU.mult)
        nc.vector.tensor_mul(dh8[:], dh8[:], dh8[:])
        var8 = stat.tile([128, NCH], F32, tag="var8", name=f"var8_{b}")
        nc.vector.tensor_add(var8[:], st8[:, :, 2], st8[:, :, 5])
        nc.vector.scalar_tensor_tensor(var8[:], var8[:], 1.0 / 128.0, dh8[:],
                                       op0=ALU.mult, op1=ALU.add)
        ln8 = stat.tile([128, NCH], F32, tag="ln8", name=f"ln8_{b}")
        nc.scalar.activation(ln8[:], var8[:], AF.Ln, bias=epst[:])
        rstd8 = stat.tile([128, NCH], F32, tag="rstd8", name=f"rstd8_{b}")
        nc.scalar.activation(rstd8[:], ln8[:], AF.Exp, scale=-0.5)
        yo = mlpp.tile([128, NCH, 128], F32, tag="yo", name=f"yo_{b}")
        for c in range(NCH):
            nc.vector.tensor_scalar(yo[:, c, :], ysum[:, c, :],
                                    mu8[:, c:c + 1], rstd8[:, c:c + 1],
                                    op0=ALU.subtract, op1=ALU.mult)
        nc.sync.dma_start(out_ext[b].rearrange("(c p) e -> p c e", p=128),
                          yo[:])

    ktb = _ktb(0)
    for h in range(H):
        eds0, zout0 = _half(h, 0, ktb)
        eds1, zout1 = _half(h, 1, ktb)
        rr0 = _rr(h, 0, zout0)
        _pv(h, 0, eds0, rr0)
        if h < H - 1:
            ktb = _ktb(h + 1)
        rr1 = _rr(h, 1, zout1)
        _pv(h, 1, eds1, rr1, and_mlp=(h == H - 1))


_NC_CACHE = None


def make_in_maps(inputs):
    import ml_dtypes

    bf = ml_dtypes.bfloat16
    f = {k: np.asarray(v, dtype=np.float32) for k, v in inputs.items()}
    Wq, Wk, Wv, W1 = f["Wq"], f["Wk"], f["Wv"], f["W1"]
    common = {
        "wqt": np.ascontiguousarray(Wq.transpose(1, 0, 2).astype(bf)),
        "wkt": np.ascontiguousarray(Wk.transpose(1, 0, 2).astype(bf)),
        # Wv is [H, E, E]; same layout as Wq with D == E
        "wvt": np.ascontiguousarray(Wv.transpose(1, 0, 2).astype(bf)),
        "bqr": np.ascontiguousarray(f["bq"][None, :, :].astype(bf)),
        "bkr": np.ascontiguousarray(f["bk"][None, :, :].astype(bf)),
        "bvr": np.ascontiguousarray(f["bv"][None, :, :].astype(bf)),
        "bkc": np.ascontiguousarray(f["bk"].transpose(1, 0)),
        "w1t": np.ascontiguousarray(
            W1.reshape(H, 128, E).transpose(1, 0, 2).astype(bf)),
        "w2c": np.ascontiguousarray(f["W2"].astype(bf)),
        "b1c": np.ascontiguousarray(f["b1"][:, None]),
        "b2r": np.ascontiguousarray(f["b2"][None, :].astype(bf)),
    }
    x = f["x"]
    in_maps = []
    for c in range(NCORES):
        m = dict(common)
        xs = np.ascontiguousarray(x[c * BL:(c + 1) * BL])
        m["x"] = xs
        m["xt"] = np.ascontiguousarray(xs.transpose(2, 0, 1).astype(bf))
        in_maps.append(m)
    return in_maps


def kernel(**inputs):
    global _NC_CACHE
    if _NC_CACHE is None:
        _NC_CACHE = _build()
    nc = _NC_CACHE

    in_maps = make_in_maps(inputs)
    res = run_bass_kernel_spmd(nc, in_maps, list(range(NCORES)))
    out = np.concatenate([res.results[c]["out"] for c in range(NCORES)],
                         axis=0).astype(np.float32)
    # final LayerNorm affine (gamma == 1, beta == 0 in practice, but apply
    # faithfully host-side)
    gamma = np.asarray(inputs["gamma"], dtype=np.float32)
    beta = np.asarray(inputs["beta"], dtype=np.float32)
    return out * gamma[None, None, :] + beta[None, None, :]


if __name__ == "__main__":
    nc = _build()
    print("built ok")
